# revision 15
# baseline (speedup 1.0000x reference)
"""DeepConvNet Trainium2 kernel.

3x [Conv3x3(pad=1) -> ReLU -> MaxPool2x2] -> Linear, N=64, input 3x128x128.

Sharding: pure data parallel, 8 images per NeuronCore across 8 cores.

Per-core dataflow (activations bf16 in SBUF, fp32 PSUM accumulation):
  conv1: im2col in partitions, 2-image pairs, block-diagonal weights:
         K = 27 taps x 2 imgs + 1 bias row = 55 partitions, M = 2 imgs x
         32 ch = 64.  Two pairs run CONCURRENTLY on the PE via tile
         positions (0,0) / (64,64) (pair A rows 0-54 -> psum 0-63, pair
         B rows 64-118 -> psum 64-127).  rhs built by strided DMAs from
         host-padded x in HBM; one DMA per (group, tap, chunk) covers
         both pairs ([[9p,6],[64p,2],[1,w]]).  Using partitions 0-118
         (vs 0-108 in one block) spreads the writes over more SDMA
         engines.  Warmup matmuls run on a memset tile so the PE is
         busy from the end of the NEFF preamble, no DMA dependency.
  bias:  conv1/conv2 fold bias into the matmul as an extra contraction
         row (ones in the rhs, bias values in the weights); ReLU folds
         into the second pool max via scalar_tensor_tensor (max(x,0)).
  pool:  PSUM can only feed one operand of a DVE op, so ScalarE copies
         even columns PSUM->SBUF, DVE maxes odd PSUM columns against the
         copy, then maxes row pairs into a zero-bordered padded tile.
  conv2: kx-replicated rhs (K = 32 ch x 3 kx + bias = 97) built into one
         [97, 8*66*66] tile by one SBUF->SBUF DMA per (image, row-half)
         ((kx, ch) merged into 96 strided partitions); halves let conv2
         start when pool1 is half done; 3 accumulated matmuls over ky;
         two images run concurrently via column tiling (img A -> array
         cols 0-63, img B -> cols 64-127).
  conv3: no replication: 9 accumulated matmuls (K=64) per image; two
         images run concurrently via row tiling (A rows 0-63, B 64-127).
         Bias+ReLU fold into the pool evacuation (two scalar-engine
         activations with per-partition bias, then DVE maxes) so no
         serial activation pass sits between conv3 and the fc.
  fc:    256 accumulated matmuls (K=128 channels, one per spatial p),
         N = 8 images, M = 10 classes, 4-way column tiling.
  sched: conv2 and conv3 are interleaved in a ladder (q0,q1,c3q0,q2,
         c3q1,q3,c3q2,c3q3) so conv3 matmuls cover the rhs2 DMA latency
         of the following conv2 pair.
"""

import os
import sys

import numpy as np

for _p in ("/opt/trn_rl_repo", "/root/.axon_site/_ro/trn_rl_repo"):
    if os.path.isdir(_p) and _p not in sys.path:
        sys.path.insert(0, _p)

import ml_dtypes

import concourse.bass as bass
import concourse.mybir as mybir
import concourse.tile as tile
from concourse import bacc
from concourse.bass_utils import run_bass_kernel_spmd

BF16 = mybir.dt.bfloat16
F32 = mybir.dt.float32
NPBF16 = ml_dtypes.bfloat16

N_CORES = 8
IMGS = 8          # images per core
GROUPS = 2        # conv1 image groups per core (4 imgs each)
G1 = 130          # conv1 padded width/height
W1WIN = 127 * G1 + 128  # flat window length per conv1 im2col row
W1ALLOC = 128 * G1
P1 = 66           # conv1 pooled padded grid (64 + 2)
P1F = 67 * 66     # pp1 alloc free size (one guard row for the kx shifts)
P2 = 34           # conv2 pooled padded grid (32 + 2)
P2F = 34 * 34
R2F = 66 * 66     # conv2 rhs bytes per image (free dim)
WARMUP_MMS = 80   # keep PE busy (and HAM warm) while im2col DMAs land

def _build_nc(dbg=False):
    nc = bacc.Bacc("TRN2", target_bir_lowering=False, debug=False)

    xp = nc.dram_tensor("xp", [IMGS * 3 * G1 * G1], BF16, kind="ExternalInput")
    lhsT1 = nc.dram_tensor("lhsT1", [128, 64], BF16, kind="ExternalInput")
    wcomb = nc.dram_tensor("wcomb", [128, 4096], BF16, kind="ExternalInput")
    wf32 = nc.dram_tensor("wf32", [128, 2], F32, kind="ExternalInput")
    ones_d = nc.dram_tensor("ones_d", [8 * R2F], BF16, kind="ExternalInput")
    scores = nc.dram_tensor("scores", [10, 8], F32, kind="ExternalOutput")

    Relu = mybir.ActivationFunctionType.Relu
    Copy = mybir.ActivationFunctionType.Copy
    MAX = mybir.AluOpType.max

    with tile.TileContext(nc) as tc:
        with (
            tc.tile_pool(name="wts", bufs=1) as wp,
            tc.tile_pool(name="rhs1", bufs=2) as rhs1p,
            tc.tile_pool(name="pp1", bufs=2) as pp1p,
            tc.tile_pool(name="rhs2", bufs=1) as rhs2p,
            tc.tile_pool(name="pp2", bufs=4) as pp2p,
            tc.tile_pool(name="xall", bufs=1) as xallp,
            tc.tile_pool(name="tmp", bufs=4) as tmpp,
            tc.tile_pool(name="ps", bufs=4, space="PSUM") as psp,
        ):
            # ---- warmup: junk matmuls with no DMA dependency keep the
            # PE busy (and warm the HAM clock gate) from the end of the
            # NEFF preamble until the first im2col chunk lands.
            t_warm = wp.tile([128, 128], BF16)
            nc.gpsimd.memset(t_warm[:], 0)
            ps_warm = psp.tile([128, 128], F32, tag="ps", name="ps_warm")
            for _ in range(WARMUP_MMS):
                nc.tensor.matmul(
                    ps_warm[:], t_warm[:], t_warm[:], start=True, stop=True
                )

            # ---- weight / constant loads
            t_l1 = wp.tile([128, 64], BF16)
            nc.sync.dma_start(out=t_l1[:], in_=lhsT1.ap())
            t_wcomb = wp.tile([128, 4096], BF16)
            nc.gpsimd.dma_start(out=t_wcomb[:], in_=wcomb.ap())
            t_l2 = t_wcomb[0:97, 0:384]
            t_l3 = t_wcomb[:, 384:1536]
            t_wfc = t_wcomb[:, 1536:4096]
            t_wf32 = wp.tile([128, 2], F32)
            nc.scalar.dma_start(out=t_wf32[:], in_=wf32.ap())
            t_b3 = t_wf32[:, 0:1]
            t_bfc = t_wf32[0:10, 1:2]

            # ---- conv1 rhs tiles (one per 4-image group; pair A in
            # partitions 0-54, pair B in 64-118, ones rows at 0 and 64)
            rhs1t = [rhs1p.tile([128, W1ALLOC], BF16, name=f"rhs1_{g}")
                     for g in range(GROUPS)]
            for g in range(GROUPS):
                for half in range(2):
                    nc.scalar.dma_start(
                        out=rhs1t[g][64 * half : 64 * half + 1, :],
                        in_=bass.AP(ones_d, 0, [[1, W1ALLOC]]),
                    )

            # ---- padded pool-output tiles: border memsets run early so
            # they never sit behind DMA waits in the gpsimd queue.
            pp1_tiles = []
            for g in range(GROUPS):
                pp1 = pp1p.tile([128, P1F], BF16, tag="pp1", name=f"pp1_{g}")
                pv = pp1.rearrange("p (r q) -> p r q", q=P1)
                # zero borders + guard row only; interior is overwritten
                nc.gpsimd.memset(pp1[:, 0:P1], 0)
                nc.gpsimd.memset(pp1[:, 65 * P1 : P1F], 0)  # bottom + guard
                nc.gpsimd.memset(pv[:, 1:65, 0:1], 0)
                nc.gpsimd.memset(pv[:, 1:65, 65:66], 0)
                pp1_tiles.append(pp1)
            pp2_tiles = []
            for q in range(4):
                pp2 = pp2p.tile([128, P2F], BF16, tag="pp2", name=f"pp2_{q}")
                pv2 = pp2.rearrange("p (r q) -> p r q", q=P2)
                nc.gpsimd.memset(pp2[:, 0:P2], 0)
                nc.gpsimd.memset(pp2[:, 33 * P2 : P2F], 0)
                nc.gpsimd.memset(pv2[:, 1:33, 0:1], 0)
                nc.gpsimd.memset(pv2[:, 1:33, 33:34], 0)
                pp2_tiles.append(pp2)

            # ---- im2col DMAs: per (chunk, group, tap, pair-half) one DMA
            # of 6 stride-9 partitions (SBUF APs may only stride
            # partitions in dim0): dst partition 64*half+1+9*(3u+c)+t.
            # g0 is column-chunked so conv1 can start early; g1 is not.
            dmas = [nc.sync, nc.scalar, nc.gpsimd]
            GCH = [[0, 4160, W1WIN], [0, W1WIN]]
            for g in range(GROUPS):
                r = rhs1t[g]
                pitch = r.ap[0][0]
                ch = GCH[g]
                for ci in range(len(ch) - 1):
                    c0 = ch[ci]
                    wlen = ch[ci + 1] - c0
                    for t in range(9):
                        a, b = divmod(t, 3)
                        for half in range(2):
                            src = bass.AP(
                                xp,
                                (4 * g + 2 * half) * 3 * G1 * G1 + a * G1 + b + c0,
                                [[G1 * G1, 6], [1, wlen]],
                            )
                            dst = bass.AP(
                                r.tensor,
                                r.offset + (64 * half + 1 + t) * pitch + c0,
                                [[9 * pitch, 6], [1, wlen]],
                            )
                            dmas[t % 3].dma_start(out=dst, in_=src)
            if dbg:
                d_rhs1 = nc.dram_tensor(
                    "d_rhs1", [128, W1ALLOC], BF16, kind="ExternalOutput"
                )
                nc.sync.dma_start(out=d_rhs1.ap(), in_=rhs1t[0][:])

            x_all = xallp.tile([128, 2048], BF16)

            def pool_psum(ps, out_ap, w, name, relu):
                """2x2 maxpool of a [128, 1024] psum tile (rows of width w)
                into out_ap (free dims (1024/w/2, w/2)); relu=True also
                clamps at 0 (valid when the bias is already in psum)."""
                psv = ps.rearrange("p (a two) -> p a two", two=2)
                cp = tmpp.tile([128, 512], F32, tag="tmpc", name=f"cp_{name}")
                nc.scalar.activation(cp[:], psv[:, :, 0], Copy)
                m1 = tmpp.tile([128, 512], BF16, tag="tmpm", name=f"m1_{name}")
                nc.vector.tensor_max(m1[:], psv[:, :, 1], cp[:])
                tv = m1.rearrange("p (y two x) -> p y two x", two=2, x=w // 2)
                if relu:
                    nc.vector.scalar_tensor_tensor(
                        out_ap, tv[:, :, 0, :], 0.0, tv[:, :, 1, :], MAX, MAX
                    )
                else:
                    nc.vector.tensor_max(out_ap, tv[:, :, 0, :], tv[:, :, 1, :])

            def pool_psum_bias_relu(ps, out_ap, w, name, bias):
                """Pool with per-partition bias + ReLU folded into the two
                scalar-engine PSUM evacuations (bias/relu commute with max)."""
                psv = ps.rearrange("p (a two) -> p a two", two=2)
                cp = tmpp.tile([128, 512], F32, tag="tmpc", name=f"cpe_{name}")
                nc.scalar.activation(cp[:], psv[:, :, 0], Relu, bias=bias)
                cp2 = tmpp.tile([128, 512], F32, tag="tmpd", name=f"cpo_{name}")
                nc.scalar.activation(cp2[:], psv[:, :, 1], Relu, bias=bias)
                m1 = tmpp.tile([128, 512], BF16, tag="tmpm", name=f"m1_{name}")
                nc.vector.tensor_max(m1[:], cp2[:], cp[:])
                tv = m1.rearrange("p (y two x) -> p y two x", two=2, x=w // 2)
                nc.vector.tensor_max(out_ap, tv[:, :, 0, :], tv[:, :, 1, :])

            # =======================  conv1  =======================
            for g in range(GROUPS):
                rv = rhs1t[g].rearrange("p (y x) -> p y x", x=G1)
                pv = pp1_tiles[g].rearrange("p (r q) -> p r q", q=P1)
                for k in range(16):
                    ps = psp.tile([128, 1024], F32, tag="ps", name=f"ps1_{g}_{k}")
                    for h in range(2):
                        y0 = k * 8 + h * 4
                        for pr in range(2):  # concurrent pair streams
                            p0 = 64 * pr
                            nc.tensor.matmul(
                                ps[p0 : p0 + 64, h * 512 : (h + 1) * 512],
                                t_l1[p0 : p0 + 55, :],
                                rv[p0 : p0 + 55, y0 : y0 + 4, 0:128],
                                start=True,
                                stop=True,
                            )
                    Y0 = k * 4
                    pool_psum(
                        ps, pv[:, Y0 + 1 : Y0 + 5, 1:65], 128, f"c1_{g}_{k}", True
                    )

            # ---- conv2 rhs DMAs: per pair one [97, 2*66*66] tile (2-slot
            # pool: pair q+2 recycles pair q's slot after its matmuls).
            # Per (image, kx) one full-frame DMA into stride-3 partitions
            # (SBUF APs may only stride partitions in dim0).
            # engines: q0/q1 -> gpsimd (the only queue that is free when
            # pool1-g0 completes), q2/q3 -> sync.  Never scalar: its
            # queue runs the pool activations and a waiting dma_start
            # would block them.
            r2eng = [nc.gpsimd, nc.gpsimd, nc.sync, nc.sync]
            r2_tiles = []
            for q in range(4):
                g, pr = divmod(q, 2)
                pp1 = pp1_tiles[g]
                p1pitch = pp1.ap[0][0]
                r2p = rhs2p.tile([97, 2 * R2F], BF16, tag="r2", name=f"r2_{q}")
                r2pitch = r2p.ap[0][0]
                r2eng[q].dma_start(
                    out=r2p[0:1, :],
                    in_=bass.AP(ones_d, 0, [[1, 2 * R2F]]),
                )
                for j in range(2):
                    i1 = pr * 2 + j
                    for kx in range(3):
                        src = bass.AP(
                            pp1.tensor,
                            (32 * i1) * p1pitch + kx,
                            [[p1pitch, 32], [1, R2F]],
                        )
                        dst = bass.AP(
                            r2p.tensor,
                            r2p.offset + (1 + kx) * r2pitch + j * R2F,
                            [[3 * r2pitch, 32], [1, R2F]],
                        )
                        r2eng[q].dma_start(out=dst, in_=src)
                r2_tiles.append(r2p)

            def conv2_pair(q):
                pv2 = pp2_tiles[q].rearrange("p (r q) -> p r q", q=P2)
                r2v = r2_tiles[q].rearrange("p (i y x) -> p i y x", i=2, x=66)
                for k in range(4):
                    ps = psp.tile([128, 1024], F32, tag="ps", name=f"ps2_{q}_{k}")
                    for h in range(2):
                        Y0 = k * 16 + h * 8
                        for ky in range(3):
                            for j in range(2):
                                rview = r2v[:, j, Y0 + ky : Y0 + ky + 8, 0:64]
                                nc.tensor.matmul(
                                    ps[64 * j : 64 * j + 64, h * 512 : (h + 1) * 512],
                                    t_l2[:, ky * 128 + 64 * j : ky * 128 + 64 * j + 64],
                                    rview,
                                    start=(ky == 0),
                                    stop=(ky == 2),
                                )
                    Y0 = k * 8
                    pool_psum(
                        ps, pv2[:, Y0 + 1 : Y0 + 9, 1:33], 64, f"c2_{q}_{k}", True
                    )

            def conv3_pair(q):
                pv2 = pp2_tiles[q].rearrange("p (r q) -> p r q", q=P2)
                ps_ab = [
                    psp.tile([128, 1024], F32, tag="ps", name=f"ps3_{q}_{jj}")
                    for jj in range(2)
                ]
                for h in range(2):
                    Y0 = h * 16
                    for t in range(9):
                        a, b = divmod(t, 3)
                        for j in range(2):  # img A (rows 0-63), img B (rows 64-127)
                            nc.tensor.matmul(
                                ps_ab[j][:, h * 512 : (h + 1) * 512],
                                t_l3[64 * j : 64 * j + 64, t * 128 : (t + 1) * 128],
                                pv2[64 * j : 64 * j + 64, Y0 + a : Y0 + a + 16, b : b + 32],
                                start=(t == 0),
                                stop=(t == 8),
                            )
                for j in range(2):
                    img = 2 * q + j
                    xv = x_all.rearrange("p (i q) -> p i q", q=256)
                    ov = xv[:, img, :].rearrange("p (y x) -> p y x", x=16)
                    pool_psum_bias_relu(ps_ab[j], ov, 32, f"c3_{q}_{j}", t_b3)

            # ladder: conv3 of earlier pairs covers rhs2 DMA latency of
            # later conv2 pairs.
            conv2_pair(0)
            conv2_pair(1)
            conv3_pair(0)
            conv2_pair(2)
            conv3_pair(1)
            conv2_pair(3)
            conv3_pair(2)
            conv3_pair(3)

            if dbg:
                d_pp1 = nc.dram_tensor("d_pp1", [128, P1F], BF16, kind="ExternalOutput")
                nc.sync.dma_start(out=d_pp1.ap(), in_=pp1_tiles[0][:])
                d_pp2 = nc.dram_tensor("d_pp2", [128, P2F], BF16, kind="ExternalOutput")
                nc.sync.dma_start(out=d_pp2.ap(), in_=pp2_tiles[0][:])
                d_xall = nc.dram_tensor("d_xall", [128, 2048], BF16, kind="ExternalOutput")
                nc.sync.dma_start(out=d_xall.ap(), in_=x_all[:])

            # =======================  fc  =======================
            ps_fc = psp.tile([128, 8], F32, tag="ps", name="ps_fc")
            xv = x_all.rearrange("p (i q) -> p i q", q=256)
            for p in range(256):
                cg = p % 4
                nc.tensor.matmul(
                    ps_fc[32 * cg : 32 * cg + 10, :],
                    t_wfc[:, 10 * p : 10 * p + 10],
                    xv[:, :, p],
                    start=(p < 4),
                    stop=(p >= 252),
                    tile_position=(0, 32 * cg),
                )
            sc0 = wp.tile([10, 8], F32)
            nc.scalar.activation(sc0[:], ps_fc[0:10, :], Copy)
            sc1 = wp.tile([10, 8], F32)
            nc.vector.tensor_add(sc1[:], ps_fc[32:42, :], sc0[:])
            sc2 = wp.tile([10, 8], F32)
            nc.vector.tensor_add(sc2[:], ps_fc[64:74, :], sc1[:])
            sc3 = wp.tile([10, 8], F32)
            nc.vector.tensor_add(sc3[:], ps_fc[96:106, :], sc2[:])
            sc = wp.tile([10, 8], F32)
            nc.scalar.activation(sc[:], sc3[:], mybir.ActivationFunctionType.Identity, bias=t_bfc)
            nc.sync.dma_start(out=scores.ap(), in_=sc[:])

    nc.compile()
    return nc


def _prep_weights(w1, b1, w2, b2, w3, b3, w_fc, b_fc):
    """Host-side weight rearrangement (shared across cores)."""
    # conv1 pair-block lhsT: per pair-half, row 64*half + 1 + u*27 + c*9
    # + t, col m = u*32 + f; rows 0/64 carry the bias (rhs ones-rows).
    l1 = np.zeros((128, 64), np.float32)
    for half in range(2):
        for u in range(2):
            for c in range(3):
                for t in range(9):
                    a, b = divmod(t, 3)
                    l1[64 * half + 1 + u * 27 + c * 9 + t, u * 32 : u * 32 + 32] = (
                        w1[:, c, a, b]
                    )
        l1[64 * half, :] = np.tile(np.asarray(b1, np.float32), 2)
    # conv2: rows p = 1 + kx + 3c, col block ky: [W2_ky | W2_ky]; bias row
    # 0 only in the ky=0 block (bias added once per accumulation).
    l2 = np.zeros((97, 3 * 128), np.float32)
    for ky in range(3):
        for kx in range(3):
            blk = w2[:, :, ky, kx].T  # [c, f]
            l2[1 + kx : 97 : 3, ky * 128 : ky * 128 + 64] = blk
            l2[1 + kx : 97 : 3, ky * 128 + 64 : ky * 128 + 128] = blk
    l2[0, 0:128] = np.tile(np.asarray(b2, np.float32), 2)
    # conv3: rows c (dup at 64+c), col block t
    l3 = np.zeros((128, 9 * 128), np.float32)
    for t in range(9):
        a, b = divmod(t, 3)
        blk = w3[:, :, a, b].T  # [c=64, f=128]
        l3[0:64, t * 128 : (t + 1) * 128] = blk
        l3[64:128, t * 128 : (t + 1) * 128] = blk
    # fc: w_fc[c*256 + p, cls] -> wfc[c, p*10 + cls]
    wf = np.ascontiguousarray(w_fc.reshape(128, 256, 10).reshape(128, 2560))
    wcomb = np.zeros((128, 4096), np.float32)
    wcomb[0:97, 0:384] = l2
    wcomb[:, 384:1536] = l3
    wcomb[:, 1536:4096] = wf
    wf32 = np.zeros((128, 2), np.float32)
    wf32[:, 0] = np.asarray(b3, np.float32)
    wf32[0:10, 1] = np.asarray(b_fc, np.float32)
    return {
        "lhsT1": l1.astype(NPBF16),
        "wcomb": wcomb.astype(NPBF16),
        "wf32": wf32,
        "ones_d": np.ones(8 * R2F, NPBF16),
    }


_NC_CACHE = {}


def get_nc():
    if "nc" not in _NC_CACHE:
        _NC_CACHE["nc"] = _build_nc()
    return _NC_CACHE["nc"]


def kernel(x, w1, b1, w2, b2, w3, b3, w_fc, b_fc, **run_kwargs):
    x = np.asarray(x, np.float32)
    wts = _prep_weights(
        np.asarray(w1, np.float32), np.asarray(b1, np.float32),
        np.asarray(w2, np.float32), np.asarray(b2, np.float32),
        np.asarray(w3, np.float32), np.asarray(b3, np.float32),
        np.asarray(w_fc, np.float32), np.asarray(b_fc, np.float32),
    )
    xpad = np.pad(x, ((0, 0), (0, 0), (1, 1), (1, 1))).astype(NPBF16)
    in_maps = []
    for core in range(N_CORES):
        m = dict(wts)
        m["xp"] = np.ascontiguousarray(xpad[core * IMGS : (core + 1) * IMGS]).reshape(-1)
        in_maps.append(m)

    nc = get_nc()
    res = run_bass_kernel_spmd(nc, in_maps, core_ids=list(range(N_CORES)), **run_kwargs)
    out = np.concatenate([r["scores"].T for r in res.results], axis=0)
    kernel.last_results = res
    return out.astype(np.float32)
